# revision 1
# baseline (speedup 1.0000x reference)
"""Trainium2 Bass kernel for a fused attention block (B=4, C=256, N=2048, H=8).

Sharding: 8 cores = 4 batches x 2 head-groups (4 heads each). Each core:
  - projects its batch's x to Q,K (stacked [4h*32d, N]) and V^T tiles
  - computes S^T = K^T Q per head in m-tiles of 128 (row-tiled on the PE so
    the four K=32 contractions run concurrently), exp on ScalarE
  - AV matmul with a ones-column appended to V^T so the softmax denominator
    comes out of the same accumulation (row 32 of the PSUM accumulator)
  - normalizes and applies its 128-column slice of w_proj
Host sums the two head-group partial projections per batch.
"""

import numpy as np

import bass_rust
import concourse.bass as bass
import concourse.mybir as mybir
import concourse.bass_utils as bass_utils
from concourse.tile import TileContext

B, C, N, H, HD = 4, 256, 2048, 8, 32
SCALE = float(HD) ** 0.5
NCORES = 8
HPC = H // 2            # heads per core (4)
NCHUNK = 512            # n (query) columns processed per chunk
NJ = N // NCHUNK        # 8
MTILES = N // 128       # 16 key/m tiles
F32 = mybir.dt.float32

# matmul dtype mode: float32r streams at full rate (1 col/cycle) on TRN2 for
# free dims >= 256; float32 runs at 1/4 rate; bfloat16 full rate, less precise.
MM_DT = mybir.dt.bfloat16


# Tensors feeding the PE must carry the matmul dtype, produced by an on-chip
# rounding op (the BIR verifier rejects raw fp32 consumed by an fp32r matmul).


def _split_sync_waits(nc, max_waits=1):
    """This walrus build rejects instructions with >1 sync wait. Move extra
    waits onto preceding same-engine NoOps (engine stalls there instead)."""
    ctr = 0
    for f in nc.m.functions:
        for bb in f.blocks:
            out = []
            for inst in bb.instructions:
                si = inst.sync_info
                if si is not None and si.on_wait and len(si.on_wait) > max_waits:
                    waits = list(si.on_wait)
                    head, keep = waits[:-max_waits], waits[-max_waits:]
                    for i in range(0, len(head), max_waits):
                        nop = bass_rust.InstNoOp(name=f"wsplit-{ctr}")
                        ctr += 1
                        nop.engine = inst.engine
                        nop.sync_info = mybir.SyncInfo(
                            on_wait=head[i:i + max_waits], on_update=[]
                        )
                        nc.register_instruction(nop, overwrite=True)
                        out.append(nop)
                    inst.sync_info = mybir.SyncInfo(
                        on_wait=keep, on_update=list(si.on_update)
                    )
                out.append(inst)
            bb.instructions = out


def build_program():
    """Build the per-core Bass program (identical SPMD on all 8 cores)."""
    sdt = MM_DT
    nc = bass.Bass()

    # Host pre-chunks c (=256) into [128, 2, .] so partition dim is 128.
    x_in = nc.dram_tensor("x_in", [128, 2 * N], sdt, kind="ExternalInput")
    wq_in = nc.dram_tensor("wq_in", [128, 256], sdt, kind="ExternalInput")
    wk_in = nc.dram_tensor("wk_in", [128, 256], sdt, kind="ExternalInput")
    wv_in = nc.dram_tensor("wv_in", [128, 256], sdt, kind="ExternalInput")
    wp_in = nc.dram_tensor("wp_in", [128, 256], sdt, kind="ExternalInput")
    y_out = nc.dram_tensor("y_out", [256, N], F32, kind="ExternalOutput")

    out_dmas = []
    tail_insts = []

    with TileContext(nc) as tc:
        with (
            tc.tile_pool(name="persist", bufs=1) as pp,
            tc.tile_pool(name="exps", bufs=10) as xp,
            tc.tile_pool(name="work", bufs=2) as wk_pool,
            tc.tile_pool(name="stp", bufs=1, space="PSUM") as stp,
            tc.tile_pool(name="accp", bufs=1, space="PSUM") as accp,
            tc.tile_pool(name="drp", bufs=2, space="DRAM") as drp,
        ):
            # ---- PE warm-up -------------------------------------------------
            # ~4us of throwaway matmuls while the input DMAs are in flight;
            # the HAM clock gate needs ~3.4us of sustained PE activity before
            # it releases the 1.2GHz throttle, so the real projection matmuls
            # start at 2.4GHz.
            wu_sb = pp.tile([128, 512], sdt)
            nc.vector.memset(wu_sb[:, :], 0.0)
            wu_ps = stp.tile([128, 1024], F32, tag="st_a", name="wu_ps")
            for i in range(20):
                nc.tensor.matmul(
                    wu_ps[:, (i % 2) * 512:(i % 2 + 1) * 512],
                    wu_sb[:, 0:128], wu_sb[:, :],
                )

            # ---- input DMAs (host supplies matmul-dtype data) ---------------
            # small weight tensors go out on SWDGE queues, x on HWDGE, so the
            # first projection matmuls aren't stuck behind the 1MB x transfer
            x_mm = pp.tile([128, 2 * N], sdt)
            w_mm = pp.tile([128, 4 * 256], sdt)
            for i, dsrc in enumerate((wq_in, wk_in, wv_in, wp_in)):
                nc.gpsimd.dma_start(w_mm[:, i * 256:(i + 1) * 256], dsrc[:, :])
            nc.sync.dma_start(x_mm[:, 0:N], x_in[:, 0:N])
            nc.sync.dma_start(x_mm[:, N:2 * N], x_in[:, N:2 * N])
            wq_sb = w_mm[:, 0:256]
            wk_sb = w_mm[:, 256:512]
            wv_sb = w_mm[:, 512:768]
            wp_sb = w_mm[:, 768:1024]

            # ---- QKV projections -------------------------------------------
            # Q_all/K_all: [128 (4h x 32d), N], contraction over c in 2 chunks.
            # Only the first halves are emitted up front; the rest interleave
            # into chunk 0 once the exp pipeline is running.
            q_sb = pp.tile([128, N], sdt)
            k_sb = pp.tile([128, N], sdt)

            def emit_qk_half(dst, wsb, half, tag):
                qp = stp.tile([128, 1024], F32, tag=tag, name="qp")
                for s in range(2):
                    col0 = half * 1024 + s * 512
                    for cc in range(2):
                        nc.tensor.matmul(
                            qp[:, s * 512:(s + 1) * 512],
                            wsb[:, cc * 128:(cc + 1) * 128],
                            x_mm[:, cc * N + col0: cc * N + col0 + 512],
                            start=(cc == 0), stop=(cc == 1),
                        )
                nc.vector.tensor_copy(dst[:, half * 1024:(half + 1) * 1024], qp[:, :])

            emit_qk_half(k_sb, wk_sb, 0, "st_a")
            emit_qk_half(q_sb, wq_sb, 0, "st_b")

            # V^T with ones column: per (head, mtile) a [128, 33] block.
            ones_f32 = pp.tile([128, 1], F32)
            nc.vector.memset(ones_f32[:, :], 1.0)
            vaug = pp.tile([128, HPC * MTILES * 33], sdt)
            nc.vector.tensor_copy(
                vaug.rearrange("p (b c) -> p b c", c=33)[:, :, 32:33],
                ones_f32[:, 0:1].to_broadcast([128, HPC * MTILES, 1]),
            )
            vaug_v = vaug.rearrange("p (h t c) -> p h t c", h=HPC, t=MTILES)

            def emit_vt_group(g):
                vp = stp.tile([128, 1024], F32, tag="st_b", name="vp")
                for mtl in range(8):
                    vmt = g * 8 + mtl
                    for cc in range(2):
                        nc.tensor.matmul(
                            vp[:, mtl * 128:(mtl + 1) * 128],
                            x_mm[:, cc * N + vmt * 128: cc * N + (vmt + 1) * 128],
                            wv_sb[:, cc * 128:(cc + 1) * 128],
                            start=(cc == 0), stop=(cc == 1),
                        )
                nc.vector.tensor_copy(
                    vaug_v[:, :, g * 8:(g + 1) * 8, 0:32],
                    vp.rearrange("p (t h d) -> p h t d", t=8, h=HPC),
                )

            def emit_exp(out_ap, in_ap):
                # manual InstActivation with immediate bias/scale: avoids the
                # per-op const-AP bias read the bass helper would emit
                eng = nc.scalar
                ins = [eng.lower_ap(in_ap),
                       mybir.ImmediateValue(dtype=F32, value=0.0),
                       mybir.ImmediateValue(dtype=F32, value=1.0 / SCALE),
                       mybir.ImmediateValue(dtype=F32, value=0.0)]
                eng.add_instruction(mybir.InstActivation(
                    name=nc.get_next_instruction_name(),
                    func=mybir.ActivationFunctionType.Exp,
                    ins=ins, outs=[eng.lower_ap(out_ap)],
                ))

            # ---- main attention loop ---------------------------------------
            # st_a holds heads 0-1 (one PSUM bank each), st_b heads 2-3; the
            # two exp ops per m-tile software-pipeline PE against ScalarE.
            AV_LAG = 3
            pending_proj = []

            def emit_proj(j, attn):
                n0 = j * NCHUNK
                yp_t = stp.tile([128, 1024], F32, tag="st_a", name="yp_t")
                for oh in range(2):
                    yp = yp_t[:, oh * 512:oh * 512 + NCHUNK]
                    nc.tensor.matmul(
                        yp[:, :],
                        wp_sb[:, oh * 128:(oh + 1) * 128],
                        attn[:, :],
                    )
                    y_sb = wk_pool.tile([128, NCHUNK], F32, tag="ysb", name="y_sb")
                    nc.vector.tensor_copy(y_sb[:, :], yp[:, :])
                    out_dmas.append(nc.sync.dma_start(
                        y_out[oh * 128:(oh + 1) * 128, n0:n0 + NCHUNK], y_sb[:, :]
                    ))

            for j in range(NJ):
                n0 = j * NCHUNK
                accs = [
                    accp.tile([128, NCHUNK], F32, tag=f"acc{h}", name=f"acc{h}")
                    for h in range(HPC)
                ]
                ex_tiles = {}
                r4 = wk_pool.tile([128, NCHUNK], F32, tag="recip", name="r4")
                nc.vector.memset(r4[:, :], 1.0)
                if j > 0:
                    # HAM keep-warm filler: the first AV_LAG iterations are
                    # S_T-only (~30% PE duty), which lets the clock gate
                    # re-throttle the PE to 1.2GHz for the rest of the chunk.
                    # Burn a few discarded matmuls into the still-idle acc
                    # banks (their accumulation groups open later).
                    for h in range(HPC):
                        nc.tensor.matmul(accs[h][:, :], wu_sb[:, 0:128], wu_sb[:, :])

                def emit_av(mt):
                    ex_mt = ex_tiles.pop(mt)
                    for h in range(HPC):
                        nc.tensor.matmul(
                            accs[h][0:33, :],
                            vaug_v[:, h, mt, :],
                            ex_mt[:, h * 512:(h + 1) * 512],
                            start=(mt == 0), stop=(mt == MTILES - 1),
                        )

                for mt in range(MTILES):
                    st_a = stp.tile([128, 1024], F32, tag="st_a", name="st_a")
                    st_b = stp.tile([128, 1024], F32, tag="st_b", name="st_b")
                    ex = xp.tile([128, 2048], sdt, name="ex")
                    ex_tiles[mt] = ex
                    for h in (0, 1):
                        nc.tensor.matmul(
                            st_a[:, h * 512:(h + 1) * 512],
                            k_sb[h * 32:(h + 1) * 32, mt * 128:(mt + 1) * 128],
                            q_sb[h * 32:(h + 1) * 32, n0:n0 + NCHUNK],
                            tile_position=(32 * h, 0),
                        )
                    emit_exp(ex[:, 0:1024], st_a[:, :])
                    for h in (2, 3):
                        nc.tensor.matmul(
                            st_b[:, (h - 2) * 512:(h - 1) * 512],
                            k_sb[h * 32:(h + 1) * 32, mt * 128:(mt + 1) * 128],
                            q_sb[h * 32:(h + 1) * 32, n0:n0 + NCHUNK],
                            tile_position=(32 * h, 0),
                        )
                    emit_exp(ex[:, 1024:2048], st_b[:, :])
                    # Deferred work interleaves here so the PE (which executes
                    # strictly in order) never parks on a not-yet-ready
                    # consumer: remaining projections early in chunk 0, the
                    # previous chunk's output projection at mt==1, and AV two
                    # m-tiles behind the exp pipeline.
                    if j == 0:
                        if mt == 1:
                            emit_qk_half(k_sb, wk_sb, 1, "st_b")
                        elif mt == 2:
                            emit_vt_group(0)
                        elif mt == 4:
                            emit_qk_half(q_sb, wq_sb, 1, "st_a")
                        elif mt == 6:
                            emit_vt_group(1)
                    if j > 0 and mt in (1, 2):
                        for h in (0, 1):
                            nc.tensor.matmul(
                                accs[2 * mt + h - 2][:, :],
                                wu_sb[:, 0:128], wu_sb[:, :],
                            )
                    if mt == 8 and pending_proj:
                        emit_proj(*pending_proj.pop())
                    if mt >= AV_LAG:
                        emit_av(mt - AV_LAG)
                for mt in range(MTILES - AV_LAG, MTILES):
                    emit_av(mt)

                # normalize this chunk. The 1/denom row is partition-broadcast
                # via a DRAM round-trip (step-0 partition APs are legal for
                # DRAM sources); the projection is deferred into the next
                # chunk so the PE queue never blocks on this chain.
                # Stage the accumulators out to SBUF immediately so the four
                # acc banks free up after ~5us and the next chunk's AV group
                # never parks the in-order PE queue (which would idle the PE
                # long enough for the HAM clock gate to re-throttle it).
                attn = wk_pool.tile([128, NCHUNK], sdt, name="attn")
                at_f = wk_pool.tile([128, NCHUNK], F32, tag="atf", name="at_f")
                for h in range(HPC):
                    nc.vector.tensor_copy(
                        r4[32 * h:32 * h + 1, :], accs[h][32:33, :]
                    )
                for h in range(HPC):
                    nc.vector.tensor_copy(
                        at_f[32 * h:(h + 1) * 32, :], accs[h][0:32, :]
                    )
                # one batched reciprocal (DVE lanes are parallel across
                # partitions; unused rows hold garbage and are never read)
                nc.vector.reciprocal(r4[:, :], r4[:, :])
                r_dram = drp.tile([4, NCHUNK], F32, name="r_dram")
                nc.sync.dma_start(
                    r_dram[:, :],
                    r4.rearrange("(a b) n -> a b n", b=32)[:, 0, :],
                )
                bc_sb = wk_pool.tile([128, NCHUNK], F32, tag="bcsb", name="bc_sb")
                for h in range(HPC):
                    nc.gpsimd.dma_start(
                        out=bc_sb[h * 32:(h + 1) * 32, :],
                        in_=r_dram[h:h + 1, :].to_broadcast([32, NCHUNK]),
                    )
                for h in range(HPC):
                    nc.vector.tensor_mul(
                        attn[h * 32:(h + 1) * 32, :],
                        at_f[h * 32:(h + 1) * 32, :], bc_sb[h * 32:(h + 1) * 32, :],
                    )
                pending_proj.append((j, attn))

            emit_proj(*pending_proj.pop())

    _split_sync_waits(nc)
    return nc


_CACHE = {}


def _get_program():
    if "nc" not in _CACHE:
        _CACHE["nc"] = build_program()
    return _CACHE["nc"]


def _core_inputs(x, w_qkv, w_proj, core):
    b, g = core // 2, core % 2
    r0 = g * 128
    wq = w_qkv[r0:r0 + 128, :].T            # [256 c, 128 (h,d)]
    wk = w_qkv[256 + r0:256 + r0 + 128, :].T
    wv = w_qkv[512 + r0:512 + r0 + 128, :].T
    wpj = w_proj[:, r0:r0 + 128].T          # [128 c_local, 256 o]

    hdt = mybir.dt.np(MM_DT)

    def chunk_c(a):  # [256, m] -> [128, 2*m] with c split across 2 free-chunks
        m = a.shape[1]
        return np.ascontiguousarray(
            a.reshape(2, 128, m).transpose(1, 0, 2).reshape(128, 2 * m)
        ).astype(hdt)

    return {
        "x_in": chunk_c(x[b]),
        "wq_in": chunk_c(wq),
        "wk_in": chunk_c(wk),
        "wv_in": chunk_c(wv),
        "wp_in": np.ascontiguousarray(wpj).astype(hdt),
    }


def kernel(x, w_qkv, w_proj, n_heads=8, _trace=False):
    x = np.asarray(x, dtype=np.float32)
    w_qkv = np.asarray(w_qkv, dtype=np.float32)
    w_proj = np.asarray(w_proj, dtype=np.float32)
    assert int(n_heads) == H

    nc = _get_program()
    in_maps = [_core_inputs(x, w_qkv, w_proj, core) for core in range(NCORES)]
    res = bass_utils.run_bass_kernel_spmd(
        nc, in_maps, core_ids=list(range(NCORES)), trace=_trace
    )
    parts = [res.results[core]["y_out"] for core in range(NCORES)]
    y = np.stack([parts[2 * b] + parts[2 * b + 1] for b in range(B)])
    if _trace:
        kernel.last_result = res
    return y.astype(np.float32)

